# revision 7
# baseline (speedup 1.0000x reference)
"""VQ codebook (nn_Codebook) Trainium2 kernel.

Data-parallel over 8 NeuronCores: each core gets 4 of the 32 batch images and
a replicated (pre-transposed, hi/lo-split) codebook.

Per-core device program:
  scores s'[n,k] = 2*z_n.e_k - ||e_k||^2 computed on TensorE in 3 passes
    (f32r hi*hi + f32r lo*hi + bf16 hi*lo) accumulated in PSUM (fp32),
  -||e||^2 folded in by an SBUF->SBUF accumulate-DMA of a broadcast row,
  argmax over k via VectorE Max8/MaxIndex (two 4096 halves),
  z_q gathered from HBM by indirect DMA with the argmax indices,
  straight-through output 2*z - z_q and the (faithful raw-reshape) loss
  partials computed on VectorE/ScalarE.

kernel(z, embedding) takes the full inputs and returns
(z_q_out [B,C,H,W] f32, min_d [B,HW] int32, loss scalar f32) like the
reference.
"""
import numpy as np
import ml_dtypes

import concourse.bass as bass
import concourse.mybir as mybir
import concourse.tile as tile
from concourse import bacc
from concourse.bass_utils import run_bass_kernel_spmd

B, C, H, W = 32, 256, 32, 32
HW = H * W
K = 8192
N_CORES = 8
B_SH = B // N_CORES          # images per core
N_TILES = HW // 128          # 8 n-tiles per image
KC = 512                     # k columns per matmul
QUART = 2048                 # psum tile width (4 banks)
BETA = 0.25

_RUNNER = None


def _f32r_round(x):
    """Round f32 array to float32r (11 explicit mantissa bits), RNE-ish."""
    b = np.asarray(x, dtype=np.float32).view(np.uint32).astype(np.int64)
    b = (b + (1 << 11)) & ~((1 << 12) - 1)
    return (b & 0xFFFFFFFF).astype(np.uint32).view(np.float32)


def _build_program(reps=1):
    nc = bacc.Bacc("TRN2", target_bir_lowering=False, debug=False,
                   num_devices=N_CORES)
    f32, f32r, bf16, u32 = (mybir.dt.float32, mybir.dt.float32r,
                            mybir.dt.bfloat16, mybir.dt.uint32)

    z_d = nc.dram_tensor("z", [B_SH, C, HW], f32, kind="ExternalInput")
    emb_d = nc.dram_tensor("emb", [K, C], f32, kind="ExternalInput")
    ehi_d = nc.dram_tensor("ehi", [2, 128, K], f32r, kind="ExternalInput")
    elo_d = nc.dram_tensor("elo", [2, 128, K], bf16, kind="ExternalInput")
    nbc_d = nc.dram_tensor("nbc", [128, K], f32, kind="ExternalInput")

    zq_d = nc.dram_tensor("zq", [B_SH, C * HW], f32, kind="ExternalOutput")
    idx_d = nc.dram_tensor("min_idx", [B_SH, N_TILES, 128], u32,
                           kind="ExternalOutput")
    loss_d = nc.dram_tensor("loss_acc", [128, 1], f32, kind="ExternalOutput")

    with tile.TileContext(nc) as tc:
        with tc.tile_pool(name="const", bufs=1) as cpool, \
             tc.tile_pool(name="zconv", bufs=1) as zpool, \
             tc.tile_pool(name="sc", bufs=2) as scpool, \
             tc.tile_pool(name="small", bufs=2) as smpool, \
             tc.tile_pool(name="outs", bufs=3) as opool, \
             tc.tile_pool(name="acc", bufs=1) as apool, \
             tc.tile_pool(name="ps", bufs=2, space="PSUM") as pspool:

            ehi_s = [cpool.tile([128, K], f32r, name=f"ehi{cc}")
                     for cc in range(2)]
            elo_s = [cpool.tile([128, K], bf16, name=f"elo{cc}")
                     for cc in range(2)]
            nbc_s = cpool.tile([128, K], f32)
            for cc in range(2):
                nc.sync.dma_start(out=ehi_s[cc][:], in_=ehi_d[cc])
                nc.sync.dma_start(out=elo_s[cc][:], in_=elo_d[cc])
            nc.sync.dma_start(out=nbc_s[:], in_=nbc_d[:])

            loss_acc = apool.tile([128, 1], f32)
            nc.vector.memset(loss_acc[:], 0.0)

            for rep, b in ((r, bb) for r in range(reps)
                           for bb in range(B_SH)):
                # --- load + split z for this image: lhsT layout [c, n] ---
                zhi = [zpool.tile([128, HW], f32r, name=f"zhi{rep}_{b}{cc}",
                                  tag=f"zhi{cc}") for cc in range(2)]
                zlo = [zpool.tile([128, HW], f32r, name=f"zlo{rep}_{b}{cc}",
                                  tag=f"zlo{cc}") for cc in range(2)]
                zhb = [zpool.tile([128, HW], bf16, name=f"zhb{rep}_{b}{cc}",
                                  tag=f"zhb{cc}") for cc in range(2)]
                for cc in range(2):
                    zc = zpool.tile([128, HW], f32, name=f"zc{rep}_{b}{cc}", tag="zc")
                    nc.sync.dma_start(out=zc[:], in_=z_d[b, cc * 128:(cc + 1) * 128, :])
                    nc.vector.tensor_copy(out=zhi[cc][:], in_=zc[:])
                    nc.vector.tensor_tensor(out=zlo[cc][:], in0=zc[:], in1=zhi[cc][:],
                                            op=mybir.AluOpType.subtract)
                    nc.vector.tensor_copy(out=zhb[cc][:], in_=zhi[cc][:])

                for t in range(N_TILES):
                    nsl = bass.ts(t, 128)
                    half_sc = []
                    for h in range(2):
                        sc = scpool.tile([128, K // 2], f32, name="sc", tag="sc")
                        half_sc.append(sc)
                        for q in range(2):
                            pst = pspool.tile([128, QUART], f32, name="pst",
                                              tag="pst")
                            k0 = h * (K // 2) + q * QUART
                            for p, (zt, et) in enumerate(
                                    ((zhi, ehi_s), (zlo, ehi_s), (zhb, elo_s))):
                                for cc in range(2):
                                    for kc in range(QUART // KC):
                                        nc.tensor.matmul(
                                            out=pst[:, bass.ts(kc, KC)],
                                            lhsT=zt[cc][:, nsl],
                                            rhs=et[cc][:, k0 + kc * KC:
                                                       k0 + (kc + 1) * KC],
                                            start=(p == 0 and cc == 0),
                                            stop=(p == 2 and cc == 1))
                            # evict quarter to SBUF (ScalarE, near PSUM)
                            nc.scalar.copy(out=sc[:, bass.ts(q, QUART)],
                                           in_=pst[:])
                            # fold in -||e||^2 via accumulate-DMA
                            nc.gpsimd.dma_start(
                                out=sc[:, bass.ts(q, QUART)],
                                in_=nbc_s[:, k0:k0 + QUART],
                                accum_op=mybir.AluOpType.add)

                    # --- argmax over the two 4096-halves ---
                    m8 = [smpool.tile([128, 8], f32, name=f"m8_{h}", tag=f"m8{h}")
                          for h in range(2)]
                    i8 = [smpool.tile([128, 8], u32, name=f"i8_{h}", tag=f"i8{h}")
                          for h in range(2)]
                    for h in range(2):
                        nc.vector.max(out=m8[h][:], in_=half_sc[h][:])
                        nc.vector.max_index(out=i8[h][:], in_max=m8[h][:],
                                            in_values=half_sc[h][:])
                    # combine: idx = m0>=m1 ? i0 : i1+4096 ; maxv = max(m0,m1)
                    ge = smpool.tile([128, 1], u32, name="ge", tag="ge")
                    nc.vector.tensor_tensor(out=ge[:], in0=m8[0][:, :1],
                                            in1=m8[1][:, :1],
                                            op=mybir.AluOpType.is_ge)
                    idx = smpool.tile([128, 1], u32, name="idx", tag="idx")
                    nc.vector.tensor_scalar(out=idx[:], in0=i8[1][:, :1],
                                            scalar1=4096, scalar2=None,
                                            op0=mybir.AluOpType.add)
                    nc.vector.copy_predicated(idx[:], ge[:], i8[0][:, :1])
                    maxv = smpool.tile([128, 1], f32, name="maxv", tag="maxv")
                    nc.vector.tensor_tensor(out=maxv[:], in0=m8[0][:, :1],
                                            in1=m8[1][:, :1],
                                            op=mybir.AluOpType.max)
                    nc.sync.dma_start(out=idx_d[b, t], in_=idx[:])

                    # --- gather z_q rows and produce outputs ---
                    zq = opool.tile([128, C], f32, name="zq", tag="zq")
                    nc.gpsimd.indirect_dma_start(
                        out=zq[:], out_offset=None, in_=emb_d[:],
                        in_offset=bass.IndirectOffsetOnAxis(ap=idx[:], axis=0))
                    zf = opool.tile([128, C], f32, name="zf", tag="zf")
                    nc.sync.dma_start(
                        out=zf[:],
                        in_=z_d[b].rearrange("c hw -> (c hw)")
                        [t * 128 * C:(t + 1) * 128 * C]
                        .rearrange("(p f) -> p f", f=C))
                    d_t = opool.tile([128, C], f32, name="d_t", tag="d_t")
                    nc.vector.tensor_tensor(out=d_t[:], in0=zq[:], in1=zf[:],
                                            op=mybir.AluOpType.subtract)
                    o_t = opool.tile([128, C], f32, name="o_t", tag="o_t")
                    nc.vector.tensor_tensor(out=o_t[:], in0=zf[:], in1=d_t[:],
                                            op=mybir.AluOpType.subtract)
                    nc.sync.dma_start(
                        out=zq_d[b][t * 128 * C:(t + 1) * 128 * C]
                        .rearrange("(p f) -> p f", f=C),
                        in_=o_t[:])
                    dsq = opool.tile([128, C], f32, name="dsq", tag="dsq")
                    nc.vector.tensor_tensor(out=dsq[:], in0=d_t[:], in1=d_t[:],
                                            op=mybir.AluOpType.mult)
                    red = opool.tile([128, 1], f32, name="red", tag="red")
                    nc.vector.tensor_reduce(out=red[:], in_=dsq[:],
                                            axis=mybir.AxisListType.X,
                                            op=mybir.AluOpType.add)
                    nc.vector.tensor_tensor(out=loss_acc[:], in0=loss_acc[:],
                                            in1=red[:],
                                            op=mybir.AluOpType.add)

            nc.sync.dma_start(out=loss_d[:], in_=loss_acc[:])
    nc.compile()
    return nc


def _get_runner():
    global _RUNNER
    if _RUNNER is None:
        _RUNNER = _build_program()
    return _RUNNER


def _prep_inputs(z, embedding):
    z = np.ascontiguousarray(np.asarray(z, dtype=np.float32))
    emb = np.ascontiguousarray(np.asarray(embedding, dtype=np.float32))
    e2t = np.ascontiguousarray((2.0 * emb).T)          # [C, K] f32
    ehi = _f32r_round(e2t)
    elo_f = (e2t - ehi).astype(np.float32)
    elo = elo_f.astype(ml_dtypes.bfloat16)
    norm = np.sum(emb.astype(np.float64) ** 2, axis=1)  # [K]
    nbc = np.ascontiguousarray(
        np.broadcast_to((-norm).astype(np.float32)[None, :], (128, K)))
    ehi = ehi.reshape(2, 128, K)
    elo = np.ascontiguousarray(elo.reshape(2, 128, K))

    zr = z.reshape(B, C, HW)
    in_maps = []
    for c in range(N_CORES):
        in_maps.append({
            "z": np.ascontiguousarray(zr[c * B_SH:(c + 1) * B_SH]),
            "emb": emb,
            "ehi": ehi,
            "elo": elo,
            "nbc": nbc,
        })
    return in_maps


def kernel(z, embedding):
    nc = _get_runner()
    in_maps = _prep_inputs(z, embedding)
    res = run_bass_kernel_spmd(nc, in_maps, list(range(N_CORES)))
    zq_parts, idx_parts, loss_sum = [], [], 0.0
    for c in range(N_CORES):
        r = res.results[c]
        zq_parts.append(r["zq"].reshape(B_SH, C, H, W))
        idx_parts.append(r["min_idx"].reshape(B_SH, HW).astype(np.int32))
        loss_sum += float(r["loss_acc"].astype(np.float64).sum())
    z_q_out = np.concatenate(zq_parts, axis=0)
    min_d = np.concatenate(idx_parts, axis=0)
    loss = np.float32((1.0 + BETA) * loss_sum / (B * C * H * W))
    return z_q_out, min_d, loss
